# revision 10
# baseline (speedup 1.0000x reference)
"""Trainium2 Bass kernel for DictionaryLearningGumbelSoftmaxMultiView.

Data-parallel over 8 NeuronCores: each core processes 512 rows of each of
the 3 views (1536 rows total), with batch-norm statistics all-reduced
across cores.  The big neg_samples gather output is synthesized on the
TensorEngine as one-hot @ dicts matmuls so HBM only sees the sequential
output write.
"""

import sys

for _p in ("/opt/trn_rl_repo", "/opt/trn_rl_repo/concourse"):
    if _p not in sys.path:
        sys.path.insert(0, _p)

from contextlib import ExitStack

import numpy as np

import concourse.bass as bass
import concourse.bacc as bacc
import concourse.tile as tile
from concourse import mybir
from concourse.bass_utils import run_bass_kernel_spmd
from concourse.masks import make_identity

f32 = mybir.dt.float32
AX = mybir.AxisListType.X
OP = mybir.AluOpType
AF = mybir.ActivationFunctionType

P = 128          # partitions
H = 768          # hidden
C = 15           # codebook size
V = 3            # views
CN = C - 1       # negatives per row (K=1)
NCORES = 8
NPV = 4096       # rows per view (global)
RPV = NPV // NCORES   # 512 rows per view per core
NT = V * RPV     # 1536 rows per core
KC = H // P      # 6 chunks of the hidden dim
TPV = RPV // P   # 4 tiles of 128 rows per view
NTILES = V * TPV  # 12 tiles per core
NTOT = V * NPV   # 12288 global rows
TEMP = 0.5
BN_EPS = 1e-5
GEPS = 1e-20
NSUB = 512       # matmul moving free-dim


def _bcast_ap(ap, parts):
    """Broadcast a DRAM AP across `parts` partitions (step-0 leading dim)."""
    return bass.AP(tensor=ap.tensor, offset=ap.offset, ap=[[0, parts]] + list(ap.ap))


def build_nc():
    nc = bacc.Bacc()

    x = nc.dram_tensor("x", [NT, H], f32, kind="ExternalInput")[:, :]
    u = nc.dram_tensor("u", [NT, C], f32, kind="ExternalInput")[:, :]
    w_t = nc.dram_tensor("w_t", [H, H], f32, kind="ExternalInput")[:, :]
    b_t = nc.dram_tensor("b_t", [H], f32, kind="ExternalInput")[:]
    bn_g = nc.dram_tensor("bn_gamma", [H], f32, kind="ExternalInput")[:]
    bn_b = nc.dram_tensor("bn_beta", [H], f32, kind="ExternalInput")[:]
    w_proj = nc.dram_tensor("w_proj", [V, C, H], f32, kind="ExternalInput")[:, :, :]
    b_proj = nc.dram_tensor("b_proj", [V, C], f32, kind="ExternalInput")[:, :]
    dicts = nc.dram_tensor("dicts", [V, C, H], f32, kind="ExternalInput")[:, :, :]

    gw_o = nc.dram_tensor("gw_o", [NT, C], f32, kind="ExternalOutput")[:, :]
    recon_o = nc.dram_tensor("recon_o", [NT, H], f32, kind="ExternalOutput")[:, :]
    logits_o = nc.dram_tensor("logits_o", [NT, C], f32, kind="ExternalOutput")[:, :]
    neg_o = nc.dram_tensor("neg_o", [NT, CN, H], f32, kind="ExternalOutput")[:, :, :]
    revp_o = nc.dram_tensor("revp_o", [NT, CN], f32, kind="ExternalOutput")[:, :]

    with tile.TileContext(nc) as tc, ExitStack() as ctx:
        const = ctx.enter_context(tc.tile_pool(name="const", bufs=1))
        ld = ctx.enter_context(tc.tile_pool(name="ld", bufs=3))
        sm = ctx.enter_context(tc.tile_pool(name="sm", bufs=3))
        psmall = ctx.enter_context(tc.tile_pool(name="psmall", bufs=4, space="PSUM"))
        pmm = ctx.enter_context(tc.tile_pool(name="pmm", bufs=2, space="PSUM"))
        dram = ctx.enter_context(tc.tile_pool(name="dram", bufs=1, space="DRAM"))

        # ---------------- constants ----------------
        identity = const.tile([P, P], f32)
        make_identity(nc, identity)

        dicts_sb = const.tile([V * C, H], f32)
        nc.sync.dma_start(dicts_sb, dicts.rearrange("v c h -> (v c) h"))
        dicts_v = []
        for v in range(V):
            dv = const.tile([C, H], f32, name=f"dicts_v{v}")
            nc.sync.dma_start(dv, dicts[v])
            dicts_v.append(dv)

        iota15 = const.tile([P, C], f32)
        nc.gpsimd.iota(iota15, pattern=[[1, C]], base=0, channel_multiplier=0,
                       allow_small_or_imprecise_dtypes=True)
        desc15 = const.tile([P, C], f32)   # C - j
        nc.vector.tensor_scalar(out=desc15, in0=iota15, scalar1=-1.0,
                                scalar2=float(C), op0=OP.mult, op1=OP.add)
        iota45r = const.tile([P, V * C], f32)   # 0..44 along free dim
        nc.gpsimd.iota(iota45r, pattern=[[1, V * C]], base=0, channel_multiplier=0,
                       allow_small_or_imprecise_dtypes=True)
        epscol = const.tile([P, 1], f32)
        nc.vector.memset(epscol, BN_EPS)
        gepscol = const.tile([P, 1], f32)
        nc.vector.memset(gepscol, GEPS)
        onecol = const.tile([P, 1], f32)
        nc.vector.memset(onecol, 1.0)
        j14v = const.tile([P, V, CN], f32)   # j + 15*v
        for v in range(V):
            nc.vector.tensor_scalar(out=j14v[:, v, :], in0=iota15[:, :CN],
                                    scalar1=float(C * v), scalar2=None, op0=OP.add)

        btc = const.tile([P, KC], f32)
        nc.sync.dma_start(btc, b_t.rearrange("(k p) -> p k", p=P))
        gmc = const.tile([P, KC], f32)
        nc.sync.dma_start(gmc, bn_g.rearrange("(k p) -> p k", p=P))
        bbc = const.tile([P, KC], f32)
        nc.sync.dma_start(bbc, bn_b.rearrange("(k p) -> p k", p=P))

        bproj_bc = const.tile([P, V, C], f32)
        nc.sync.dma_start(bproj_bc, _bcast_ap(b_proj, P))

        # w_proj[v].T per view / hidden chunk: wpT[:, v, k, :] = w_proj[v, :, k*P+p].T
        wpT = const.tile([P, V, KC, C], f32)
        for v in range(V):
            wp = ld.tile([C, H], f32, tag="wp")
            nc.sync.dma_start(wp, w_proj[v])
            for k in range(KC):
                ps = psmall.tile([P, C], f32, tag="tp")
                nc.tensor.transpose(ps, wp[:, k * P:(k + 1) * P], identity[:C, :C])
                nc.vector.tensor_copy(out=wpT[:, v, k, :], in_=ps)

        # ---------------- phases A-C in a scope whose pools free before D ----
        hT_pool = ctx.enter_context(tc.tile_pool(name="hT", bufs=1))
        hTt = [hT_pool.tile([P, NT], f32, tag=f"hT{k}", name=f"hT{k}") for k in range(KC)]
        ssum = const.tile([P, 2 * KC], f32)

        with ExitStack() as phase_ab:
            bld = phase_ab.enter_context(tc.tile_pool(name="bld", bufs=3))
            xT_pool = phase_ab.enter_context(tc.tile_pool(name="xT", bufs=1))
            wT_pool = phase_ab.enter_context(tc.tile_pool(name="wT", bufs=1))

            # phase A: transposes of x and w_t
            xTt = [xT_pool.tile([P, NT], f32, tag=f"xT{k}", name=f"xT{k}") for k in range(KC)]
            for t in range(NTILES):
                xt = bld.tile([P, H], f32, tag="x")
                nc.sync.dma_start(xt, x[t * P:(t + 1) * P, :])
                for k in range(KC):
                    ps = psmall.tile([P, P], f32, tag="tp")
                    nc.tensor.transpose(ps, xt[:, k * P:(k + 1) * P], identity)
                    nc.vector.tensor_copy(out=xTt[k][:, t * P:(t + 1) * P], in_=ps)

            wTt = [wT_pool.tile([P, H], f32, tag=f"wT{k}", name=f"wT{k}") for k in range(KC)]
            for r in range(KC):
                wt = bld.tile([P, H], f32, tag="w")
                nc.sync.dma_start(wt, w_t[r * P:(r + 1) * P, :])
                for k in range(KC):
                    ps = psmall.tile([P, P], f32, tag="tp")
                    nc.tensor.transpose(ps, wt[:, k * P:(k + 1) * P], identity)
                    nc.vector.tensor_copy(out=wTt[k][:, r * P:(r + 1) * P], in_=ps)

            # phase B: hT = w_t @ x.T + b_t
            for hc in range(KC):
                for ns in range(NT // NSUB):
                    pm = pmm.tile([P, NSUB], f32, tag="mm")
                    for k in range(KC):
                        nc.tensor.matmul(pm, wTt[k][:, hc * P:(hc + 1) * P],
                                         xTt[k][:, ns * NSUB:(ns + 1) * NSUB],
                                         start=(k == 0), stop=(k == KC - 1))
                    nc.scalar.activation(out=hTt[hc][:, ns * NSUB:(ns + 1) * NSUB],
                                         in_=pm, func=AF.Identity,
                                         bias=btc[:, hc:hc + 1], scale=1.0)

            # phase C: local batch-norm sums
            sq = bld.tile([P, NT], f32, tag="sq", bufs=1)
            for hc in range(KC):
                nc.vector.reduce_sum(out=ssum[:, hc:hc + 1], in_=hTt[hc], axis=AX)
                nc.scalar.activation(out=sq, in_=hTt[hc], func=AF.Square,
                                     accum_out=ssum[:, KC + hc:KC + hc + 1])
        cin = dram.tile([P, 2 * KC], f32)
        cout = dram.tile([P, 2 * KC], f32)
        nc.gpsimd.dma_start(cin, ssum)
        nc.gpsimd.collective_compute(
            "AllReduce", OP.add,
            replica_groups=[list(range(NCORES))],
            ins=[cin.opt()], outs=[cout.opt()],
        )
        gsum = const.tile([P, 2 * KC], f32)
        nc.sync.dma_start(gsum, cout)

        mu = const.tile([P, KC], f32)
        nc.vector.tensor_scalar_mul(mu, gsum[:, :KC], 1.0 / NTOT)
        ex2 = const.tile([P, KC], f32)
        nc.vector.tensor_scalar_mul(ex2, gsum[:, KC:], 1.0 / NTOT)
        musq = const.tile([P, KC], f32)
        nc.vector.tensor_mul(musq, mu, mu)
        var = const.tile([P, KC], f32)
        nc.vector.tensor_sub(var, ex2, musq)
        sd = const.tile([P, KC], f32)
        nc.scalar.activation(out=sd, in_=var, func=AF.Sqrt, bias=epscol)
        rsig = const.tile([P, KC], f32)
        nc.vector.reciprocal(rsig, sd)
        aco = const.tile([P, KC], f32)
        nc.vector.tensor_mul(aco, gmc, rsig)
        tmid = const.tile([P, KC], f32)
        nc.vector.tensor_mul(tmid, mu, aco)
        bco = const.tile([P, KC], f32)
        nc.vector.tensor_sub(bco, bbc, tmid)
        # h = relu(h * a + b), fused on ACT
        for hc in range(KC):
            nc.scalar.activation(out=hTt[hc], in_=hTt[hc], func=AF.Relu,
                                 scale=aco[:, hc:hc + 1], bias=bco[:, hc:hc + 1])

        # ---------------- phase D: per 128-row tile ----------------
        stage = ctx.enter_context(tc.tile_pool(name="stage", bufs=2))
        outsm = ctx.enter_context(tc.tile_pool(name="outsm", bufs=1))
        gw_st = outsm.tile([P, NTILES, C], f32)
        lg_st = outsm.tile([P, NTILES, C], f32)
        rp_st = outsm.tile([P, NTILES, CN], f32)

        for t in range(NTILES):
            v = t // TPV
            r0 = t * P

            # logits
            pl = psmall.tile([P, C], f32, tag="tp")
            for k in range(KC):
                nc.tensor.matmul(pl, hTt[k][:, r0:r0 + P], wpT[:, v, k, :],
                                 start=(k == 0), stop=(k == KC - 1))
            z = lg_st[:, t, :]
            nc.vector.tensor_add(z, pl, bproj_bc[:, v, :])

            # gumbel noise and gumbel softmax weights
            ut = ld.tile([P, C], f32, tag="u")
            nc.sync.dma_start(ut, u[r0:r0 + P, :])
            ln1 = sm.tile([P, C], f32, tag="ln1")
            nc.scalar.activation(out=ln1, in_=ut, func=AF.Ln, bias=gepscol)
            ln2 = sm.tile([P, C], f32, tag="ln2")
            nc.scalar.activation(out=ln2, in_=ln1, func=AF.Ln, scale=-1.0,
                                 bias=gepscol)
            y = sm.tile([P, C], f32, tag="y")
            nc.vector.tensor_sub(y, z, ln2)

            m = sm.tile([P, 1], f32, tag="m")
            nc.vector.reduce_max(m, y, axis=AX)
            mneg = sm.tile([P, 1], f32, tag="mneg")
            nc.vector.tensor_scalar_mul(mneg, m, -1.0 / TEMP)
            e = sm.tile([P, C], f32, tag="e")
            s = sm.tile([P, 1], f32, tag="s")
            nc.scalar.activation(out=e, in_=y, func=AF.Exp, scale=1.0 / TEMP,
                                 bias=mneg, accum_out=s)
            r = sm.tile([P, 1], f32, tag="r")
            nc.vector.reciprocal(r, s)
            gw = gw_st[:, t, :]
            nc.vector.tensor_scalar_mul(gw, e, r)

            # plain softmax -> rev = (1-probs)^2 -> exp(rev)
            m2 = sm.tile([P, 1], f32, tag="m2")
            nc.vector.reduce_max(m2, z, axis=AX)
            m2n = sm.tile([P, 1], f32, tag="m2n")
            nc.vector.tensor_scalar_mul(m2n, m2, -1.0)
            e2 = sm.tile([P, C], f32, tag="e2")
            s2 = sm.tile([P, 1], f32, tag="s2")
            nc.scalar.activation(out=e2, in_=z, func=AF.Exp, bias=m2n,
                                 accum_out=s2)
            r2 = sm.tile([P, 1], f32, tag="r2")
            nc.vector.reciprocal(r2, s2)
            probs = sm.tile([P, C], f32, tag="probs")
            nc.vector.tensor_scalar_mul(probs, e2, r2)
            rev = sm.tile([P, C], f32, tag="rev")
            nc.scalar.activation(out=rev, in_=probs, func=AF.Square, scale=-1.0,
                                 bias=onecol)
            e3 = sm.tile([P, C], f32, tag="e3")
            nc.scalar.activation(out=e3, in_=rev, func=AF.Exp)

            # top-1 mask and index
            pm_t = sm.tile([P, C], f32, tag="pm")
            nc.vector.tensor_scalar(pm_t, in0=y, scalar1=m, scalar2=None,
                                    op0=OP.is_ge)
            notpm = sm.tile([P, C], f32, tag="npm")
            nc.vector.tensor_scalar(notpm, in0=pm_t, scalar1=-1.0, scalar2=1.0,
                                    op0=OP.mult, op1=OP.add)
            e3z = sm.tile([P, C], f32, tag="e3z")
            nc.vector.tensor_mul(e3z, e3, notpm)
            s3 = sm.tile([P, 1], f32, tag="s3")
            nc.vector.reduce_sum(s3, e3z, axis=AX)
            r3 = sm.tile([P, 1], f32, tag="r3")
            nc.vector.reciprocal(r3, s3)
            full = sm.tile([P, C], f32, tag="full")
            nc.vector.tensor_scalar_mul(full, e3z, r3)

            pv = sm.tile([P, C], f32, tag="pv")
            nc.vector.tensor_mul(pv, pm_t, desc15)
            pmax = sm.tile([P, 1], f32, tag="pmax")
            nc.vector.reduce_max(pmax, pv, axis=AX)
            p_col = sm.tile([P, 1], f32, tag="pcol")
            nc.vector.tensor_scalar(p_col, in0=pmax, scalar1=-1.0,
                                    scalar2=float(C), op0=OP.mult, op1=OP.add)

            # rev_probs: drop the positive column (j < p ? full[j] : full[j+1])
            mlt = sm.tile([P, CN], f32, tag="mlt")
            nc.vector.tensor_scalar(mlt, in0=iota15[:, :CN], scalar1=p_col,
                                    scalar2=None, op0=OP.is_lt)
            fdiff = sm.tile([P, CN], f32, tag="fdiff")
            nc.vector.tensor_sub(fdiff, full[:, :CN], full[:, 1:C])
            fsel = sm.tile([P, CN], f32, tag="fsel")
            nc.vector.tensor_mul(fsel, fdiff, mlt)
            nc.vector.tensor_add(rp_st[:, t, :], fsel, full[:, 1:C])

            # negative indices g = 15*v + j + (j >= p), one-hot, gather matmul
            ge_t = sm.tile([P, CN], f32, tag="ge")
            nc.vector.tensor_scalar(ge_t, in0=iota15[:, :CN], scalar1=p_col,
                                    scalar2=None, op0=OP.is_ge)
            gplus = sm.tile([P, CN], f32, tag="gp")
            nc.vector.tensor_add(gplus, ge_t, j14v[:, v, :])

            stg = stage.tile([P, CN, H], f32, tag="stg")
            for j in range(CN):
                # one-hot in [n, r] layout: ohn[n, r] = (r == g[n, j])
                ohn = sm.tile([P, V * C], f32, tag="ohn")
                nc.vector.tensor_scalar(ohn, in0=iota45r,
                                        scalar1=gplus[:, j:j + 1],
                                        scalar2=None, op0=OP.is_equal)
                pohT = psmall.tile([V * C, P], f32, tag="tp")
                nc.tensor.transpose(pohT, ohn, identity)
                oh = sm.tile([V * C, P], f32, tag="oh")
                nc.vector.tensor_copy(out=oh, in_=pohT)
                pn = pmm.tile([P, H], f32, tag="mm")
                nc.tensor.matmul(pn[:, 0:NSUB], oh, dicts_sb[:, 0:NSUB])
                nc.tensor.matmul(pn[:, NSUB:H], oh, dicts_sb[:, NSUB:H])
                nc.any.tensor_copy(out=stg[:, j, :], in_=pn)
            nc.sync.dma_start(neg_o[r0:r0 + P, :, :], stg)

            # recon = gw @ dicts[v]
            pgw = psmall.tile([C, P], f32, tag="tp")
            nc.tensor.transpose(pgw, gw, identity)
            gwT = sm.tile([C, P], f32, tag="gwT")
            nc.vector.tensor_copy(out=gwT, in_=pgw)
            pr = pmm.tile([P, H], f32, tag="mm")
            nc.tensor.matmul(pr[:, 0:NSUB], gwT, dicts_v[v][:, 0:NSUB])
            nc.tensor.matmul(pr[:, NSUB:H], gwT, dicts_v[v][:, NSUB:H])
            rst = stage.tile([P, H], f32, tag="rst")
            nc.any.tensor_copy(out=rst, in_=pr)
            nc.sync.dma_start(recon_o[r0:r0 + P, :], rst)

        # small outputs, one DMA each
        nc.sync.dma_start(gw_o.rearrange("(t p) c -> p t c", p=P), gw_st)
        nc.sync.dma_start(logits_o.rearrange("(t p) c -> p t c", p=P), lg_st)
        nc.sync.dma_start(revp_o.rearrange("(t p) c -> p t c", p=P), rp_st)

    nc.compile()
    return nc


_NC = None


def _get_nc():
    global _NC
    if _NC is None:
        _NC = build_nc()
    return _NC


def _shard_rows(arr):
    """[NTOT, ...] -> per-core [NT, ...]: core c gets rows c*512..(c+1)*512 of
    each view segment."""
    a = np.ascontiguousarray(np.asarray(arr))
    a = a.reshape(V, NCORES, RPV, *a.shape[1:])
    return [np.ascontiguousarray(a[:, c].reshape(NT, *a.shape[3:]))
            for c in range(NCORES)]


def _unshard_rows(parts):
    """Inverse of _shard_rows."""
    rest = parts[0].shape[1:]
    out = np.empty((V, NCORES, RPV, *rest), dtype=parts[0].dtype)
    for c, p in enumerate(parts):
        out[:, c] = p.reshape(V, RPV, *rest)
    return out.reshape(NTOT, *rest)


def kernel(x, u, w_t, b_t, bn_gamma, bn_beta, w_proj, b_proj, dicts,
           n_per_view=None, **_ignored):
    nc = _get_nc()
    xs = _shard_rows(x)
    us = _shard_rows(u)
    rep = {
        "w_t": np.ascontiguousarray(np.asarray(w_t, dtype=np.float32)),
        "b_t": np.ascontiguousarray(np.asarray(b_t, dtype=np.float32)),
        "bn_gamma": np.ascontiguousarray(np.asarray(bn_gamma, dtype=np.float32)),
        "bn_beta": np.ascontiguousarray(np.asarray(bn_beta, dtype=np.float32)),
        "w_proj": np.ascontiguousarray(np.asarray(w_proj, dtype=np.float32)),
        "b_proj": np.ascontiguousarray(np.asarray(b_proj, dtype=np.float32)),
        "dicts": np.ascontiguousarray(np.asarray(dicts, dtype=np.float32)),
    }
    in_maps = [dict(rep, x=xs[c], u=us[c]) for c in range(NCORES)]
    res = run_bass_kernel_spmd(nc, in_maps, core_ids=list(range(NCORES)))
    outs = res.results
    gw = _unshard_rows([outs[c]["gw_o"] for c in range(NCORES)])
    recon = _unshard_rows([outs[c]["recon_o"] for c in range(NCORES)])
    logits = _unshard_rows([outs[c]["logits_o"] for c in range(NCORES)])
    neg = _unshard_rows([outs[c]["neg_o"] for c in range(NCORES)])
    revp = _unshard_rows([outs[c]["revp_o"] for c in range(NCORES)])
    return gw, recon, logits, neg, revp
